# revision 6
# baseline (speedup 1.0000x reference)
import sys
if "/opt/trn_rl_repo" not in sys.path:
    sys.path.insert(0, "/opt/trn_rl_repo")
import numpy as np
import concourse.bass as bass
from concourse import bacc
import concourse.tile as tile
from concourse import mybir

F32 = mybir.dt.float32
F32R = mybir.dt.float32r
BF16 = mybir.dt.bfloat16
AF = mybir.ActivationFunctionType
ALU = mybir.AluOpType

D = 512
H = 8
HD = 64
L = 2
IN = 16
S = 1024
BL = 2          # batch elems per core
NCORES = 8
LN_EPS = 1e-5
DELTA_SCALE = 1.5
NEG = -1.0e30


def _build(gates):
    """Build the per-core SPMD program. gates: dict of host-value-dependent flags."""
    import os as _os
    gates = dict(gates)
    gates["stage"] = int(_os.environ.get("KSTAGE", "9"))
    nc = bacc.Bacc(None, target_bir_lowering=False, debug=False, num_devices=NCORES)

    dp = nc.declare_dram_parameter
    featT_e = dp("featT", [BL, IN, S], F32, isOutput=False)
    hwT_e = dp("hwT", [IN, D], F32, isOutput=False)
    qwT_e = dp("qwT", [IN, D], F32, isOutput=False)
    hb_e = dp("hb", [128, 4], F32, isOutput=False)
    qb_e = dp("qb", [128, 4], F32, isOutput=False)
    wqT_e = dp("wqT", [128, L, 4, D], F32, isOutput=False)
    wkT_e = dp("wkT", [128, L, 4, D], F32, isOutput=False)
    wvT_e = dp("wvT", [128, L, 4, D], F32, isOutput=False)
    woT_e = dp("woT", [128, L, 4, D], F32, isOutput=False)
    qbl_e = dp("qbl", [128, L, 4], F32, isOutput=False)
    kbl_e = dp("kbl", [128, L, 4], F32, isOutput=False)
    vbl_e = dp("vbl", [128, L, 4], F32, isOutput=False)
    abl_e = dp("abl", [128, L, 4], F32, isOutput=False)
    lng_e = dp("lng", [128, L, 4], F32, isOutput=False)
    lnb_e = dp("lnb", [128, L, 4], F32, isOutput=False)
    opw_e = dp("opw", [128, 4], F32, isOutput=False)
    opb_e = dp("opb", [1, 1], F32, isOutput=False)
    maskA_e = dp("maskA", [128, 128], F32, isOutput=False)
    cones_e = dp("cones", [128, 128], F32, isOutput=False)
    out_e = dp("out", [BL, S], F32, isOutput=True)

    with tile.TileContext(nc) as tc:
        _emit(nc, tc, gates, dict(
            featT=featT_e, hwT=hwT_e, qwT=qwT_e, hb=hb_e, qb=qb_e,
            wqT=wqT_e, wkT=wkT_e, wvT=wvT_e, woT=woT_e,
            qbl=qbl_e, kbl=kbl_e, vbl=vbl_e, abl=abl_e,
            lng=lng_e, lnb=lnb_e, opw=opw_e, opb=opb_e,
            maskA=maskA_e, cones=cones_e, out=out_e))
    nc.compile()
    return nc


def _emit(nc, tc, gates, E):
    from contextlib import ExitStack
    ctx = ExitStack()
    with ctx:
        P = bass.MemorySpace.PSUM
        wp = ctx.enter_context(tc.tile_pool(name="wp", bufs=1))
        feat_p = ctx.enter_context(tc.tile_pool(name="feat", bufs=1))
        hist_p = ctx.enter_context(tc.tile_pool(name="hist", bufs=1))
        x_p = ctx.enter_context(tc.tile_pool(name="x", bufs=1))
        y_p = ctx.enter_context(tc.tile_pool(name="y", bufs=1))
        q_p = ctx.enter_context(tc.tile_pool(name="q", bufs=1))
        k_p = ctx.enter_context(tc.tile_pool(name="k", bufs=1))
        v_p = ctx.enter_context(tc.tile_pool(name="v", bufs=1))
        pr_p = ctx.enter_context(tc.tile_pool(name="pr", bufs=1))
        o_p = ctx.enter_context(tc.tile_pool(name="o", bufs=1))
        x2_p = ctx.enter_context(tc.tile_pool(name="x2", bufs=1))
        tmp_p = ctx.enter_context(tc.tile_pool(name="tmp", bufs=1))
        bc_p = ctx.enter_context(tc.tile_pool(name="bc", bufs=1))
        sinv_p = ctx.enter_context(tc.tile_pool(name="sinv", bufs=1))
        row_p = ctx.enter_context(tc.tile_pool(name="row", bufs=1))
        psA = ctx.enter_context(tc.tile_pool(name="psA", bufs=2, space=P))
        psS = ctx.enter_context(tc.tile_pool(name="psS", bufs=2, space=P))
        psV = ctx.enter_context(tc.tile_pool(name="psV", bufs=2, space=P))
        psB = ctx.enter_context(tc.tile_pool(name="psB", bufs=2, space=P))

        # ---- persistent weights/consts ----
        hwT = wp.tile([IN, D], F32R)
        qwT = wp.tile([IN, D], F32R)
        hb = wp.tile([128, 4], F32)
        qb = wp.tile([128, 4], F32)
        wqT = wp.tile([128, L, 4, D], F32R)
        wkT = wp.tile([128, L, 4, D], F32R)
        wvT = wp.tile([128, L, 4, D], F32R)
        woT = wp.tile([128, L, 4, D], F32R)
        qbl = wp.tile([128, L, 4], F32)
        kbl = wp.tile([128, L, 4], F32)
        vbl = wp.tile([128, L, 4], F32)
        abl = wp.tile([128, L, 4], F32)
        lng = wp.tile([128, L, 4], F32)
        lnb = wp.tile([128, L, 4], F32)
        opw = wp.tile([128, 4], F32R)
        opb = wp.tile([1, 1], F32)
        maskA = wp.tile([128, 128], F32)
        ones64r = wp.tile([1, HD], F32R)    # bcast lhsT across 64 parts
        ones128c = wp.tile([128, 1], F32R)  # LN-sum lhsT
        ones128r = wp.tile([1, 128], F32R)  # bcast lhsT across 128 parts
        cones = wp.tile([128, HD], F32)     # f32 ones for v-aug column

        g = nc.gpsimd
        g.dma_start(hwT[:], E["hwT"][:])
        g.dma_start(qwT[:], E["qwT"][:])
        g.dma_start(hb[:], E["hb"][:])
        g.dma_start(qb[:], E["qb"][:])
        g.dma_start(wqT[:], E["wqT"][:])
        g.dma_start(wkT[:], E["wkT"][:])
        g.dma_start(wvT[:], E["wvT"][:])
        g.dma_start(woT[:], E["woT"][:])
        g.dma_start(qbl[:], E["qbl"][:])
        g.dma_start(kbl[:], E["kbl"][:])
        g.dma_start(vbl[:], E["vbl"][:])
        g.dma_start(abl[:], E["abl"][:])
        g.dma_start(lng[:], E["lng"][:])
        g.dma_start(lnb[:], E["lnb"][:])
        g.dma_start(opw[:], E["opw"][:])
        g.dma_start(opb[:], E["opb"][:])
        g.dma_start(maskA[:], E["maskA"][:])
        g.dma_start(ones64r[:], E["cones"][0:1, 0:HD])
        g.dma_start(ones128c[:], E["cones"][:, 0:1])
        g.dma_start(ones128r[:], E["cones"][0:1, :])
        g.dma_start(cones[:], E["cones"][:, 0:HD])

        for b in range(BL):
            featT = feat_p.tile([IN, S], F32R)
            g.dma_start(featT[:], E["featT"][b])

            histT = hist_p.tile([128, 4, S], F32R)
            xT = x_p.tile([128, 4, S], F32R)
            # input projections: histT/xT [d, t] = W[d,:] @ featT
            for dt in range(4):
                for qs in range(2):
                    cols = bass.ts(qs, 512)
                    ps = psA.tile([128, 512], F32, tag="a")
                    nc.tensor.matmul(ps[:], hwT[:, dt * 128:(dt + 1) * 128],
                                     featT[:, cols], start=True, stop=True)
                    nc.scalar.activation(histT[:, dt, cols], ps[:], AF.Identity,
                                         bias=hb[:, dt:dt + 1])
                    ps2 = psA.tile([128, 512], F32, tag="a")
                    nc.tensor.matmul(ps2[:], qwT[:, dt * 128:(dt + 1) * 128],
                                     featT[:, cols], start=True, stop=True)
                    nc.scalar.activation(xT[:, dt, cols], ps2[:], AF.Identity,
                                         bias=qb[:, dt:dt + 1])

            if gates["stage"] < 2:
                continue
            for l in range(L):
                # ---- q/k projections (transposed layout, bf16 out) ----
                qT = q_p.tile([128, 4, S], BF16)
                kT = k_p.tile([128, 4, S], BF16)
                for dt in range(4):
                    for qs in range(2):
                        cols = bass.ts(qs, 512)
                        ps = psA.tile([128, 512], F32, tag="a")
                        for kt in range(4):
                            nc.tensor.matmul(
                                ps[:], wqT[:, l, kt, dt * 128:(dt + 1) * 128],
                                xT[:, kt, cols], start=(kt == 0), stop=(kt == 3))
                        nc.scalar.activation(qT[:, dt, cols], ps[:], AF.Identity,
                                             bias=qbl[:, l, dt:dt + 1])
                        ps2 = psA.tile([128, 512], F32, tag="a")
                        for kt in range(4):
                            nc.tensor.matmul(
                                ps2[:], wkT[:, l, kt, dt * 128:(dt + 1) * 128],
                                histT[:, kt, cols], start=(kt == 0), stop=(kt == 3))
                        nc.scalar.activation(kT[:, dt, cols], ps2[:], AF.Identity,
                                             bias=kbl[:, l, dt:dt + 1])

                # ---- v projection (natural layout + ones column, bf16) ----
                vN = v_p.tile([128, 8, H, HD + 1], BF16)
                for tt in range(8):
                    ps = psA.tile([128, 512], F32, tag="a")
                    for kt in range(4):
                        nc.tensor.matmul(
                            ps[:], histT[:, kt, tt * 128:(tt + 1) * 128],
                            wvT[:, l, kt, :], start=(kt == 0), stop=(kt == 3))
                    for h in range(H):
                        nc.scalar.copy(vN[:, tt, h, 0:HD], ps[:, h * HD:(h + 1) * HD])
                nc.scalar.copy(
                    vN[:, :, :, HD].rearrange("p a b -> p (a b)"), cones[:])

                # ---- attention ----
                if gates["stage"] < 3:
                    continue
                outT = o_p.tile([128, 4, S], F32R)
                for h in range(H):
                    hp = (h % 2) * 64
                    dht = h // 2
                    for qblk in range(2):
                        probsT = pr_p.tile([128, 8, 512], BF16)
                        pv = psV.tile([65, 512], F32, tag="v")
                        nkj = 4 * (qblk + 1)
                        for kj in range(nkj):
                            off = max(0, (kj - 4 * qblk) * 128)
                            n = 512 - off
                            sc = psS.tile([128, 512], F32, tag="s")
                            nc.tensor.matmul(
                                sc[:, off:],
                                kT[hp:hp + 64, dht, kj * 128:(kj + 1) * 128],
                                qT[hp:hp + 64, dht, qblk * 512 + off:(qblk + 1) * 512],
                                start=True, stop=True)
                            if kj >= 4 * qblk:
                                nc.vector.tensor_add(sc[:, off:off + 128],
                                                     sc[:, off:off + 128], maskA[:])
                            nc.scalar.activation(probsT[:, kj, off:], sc[:, off:],
                                                 AF.Exp, scale=0.125)
                            nc.tensor.matmul(pv[:, off:], vN[:, kj, h, :],
                                             probsT[:, kj, off:],
                                             start=(kj == 0), stop=(kj == nkj - 1))
                        # normalize by softmax sums (row 64 of pv)
                        srow = row_p.tile([1, 512], F32R, bufs=2)
                        nc.scalar.copy(srow[:], pv[64:65, :])
                        sb = psB.tile([64, 512], F32, tag="b")
                        nc.tensor.matmul(sb[:], ones64r[:], srow[:],
                                         start=True, stop=True)
                        sinv = sinv_p.tile([64, 512], F32)
                        nc.vector.reciprocal(sinv[:], sb[:])
                        cols = bass.ts(qblk, 512)
                        nc.vector.tensor_mul(outT[hp:hp + 64, dht, cols],
                                             pv[0:64, :], sinv[:])
                        if gates["vb"]:
                            nc.vector.tensor_scalar_add(
                                outT[hp:hp + 64, dht, cols],
                                outT[hp:hp + 64, dht, cols],
                                vbl[hp:hp + 64, l, dht:dht + 1])

                # ---- attn out proj + residual add ----
                if gates["stage"] < 4:
                    continue
                yT = y_p.tile([128, 4, S], F32R)
                for dt in range(4):
                    for qs in range(2):
                        cols = bass.ts(qs, 512)
                        ps = psA.tile([128, 512], F32, tag="a")
                        for kt in range(4):
                            nc.tensor.matmul(
                                ps[:], woT[:, l, kt, dt * 128:(dt + 1) * 128],
                                outT[:, kt, cols], start=(kt == 0), stop=(kt == 3))
                        nc.vector.scalar_tensor_tensor(
                            yT[:, dt, cols], ps[:], abl[:, l, dt:dt + 1],
                            xT[:, dt, cols].bitcast(F32),
                            op0=ALU.add, op1=ALU.add)

                # ---- layernorm ----
                xT = x_p.tile([128, 4, S], F32R)
                for qs in range(2):
                    cols = bass.ts(qs, 512)
                    mps = psS.tile([1, 512], F32, tag="s", padded_shape=None)
                    for dt in range(4):
                        nc.tensor.matmul(mps[:], ones128c[:], yT[:, dt, cols],
                                         start=(dt == 0), stop=(dt == 3))
                    vps = psS.tile([1, 512], F32, tag="s", padded_shape=None)
                    for dt in range(4):
                        x2 = x2_p.tile([128, 512], F32R)
                        nc.scalar.activation(x2[:], yT[:, dt, cols].bitcast(F32),
                                             AF.Square)
                        nc.tensor.matmul(vps[:], ones128c[:], x2[:],
                                         start=(dt == 0), stop=(dt == 3))
                    mrow = row_p.tile([1, 512], F32R)
                    nc.scalar.mul(mrow[:], mps[:], 1.0 / D)
                    s1 = row_p.tile([1, 512], F32)
                    nc.vector.tensor_mul(s1[:], mrow[:].bitcast(F32),
                                         mrow[:].bitcast(F32))
                    s2 = row_p.tile([1, 512], F32)
                    nc.vector.scalar_tensor_tensor(
                        s2[:], vps[:], 1.0 / D, s1[:],
                        op0=ALU.mult, op1=ALU.subtract)
                    s4 = row_p.tile([1, 512], F32)
                    nc.vector.tensor_scalar_add(s4[:], s2[:], LN_EPS)
                    s3 = row_p.tile([1, 512], F32)
                    nc.scalar.sqrt(s3[:], s4[:])
                    rrowr = row_p.tile([1, 512], F32R)
                    with nc.allow_low_precision(reason="f32r rstd"):
                        nc.vector.reciprocal(rrowr[:], s3[:])
                    # broadcast mean and rstd to 128 partitions
                    mbps = psB.tile([128, 512], F32, tag="b")
                    nc.tensor.matmul(mbps[:], ones128r[:], mrow[:],
                                     start=True, stop=True)
                    mbc = bc_p.tile([128, 512], F32)
                    nc.scalar.copy(mbc[:], mbps[:])
                    rbps = psB.tile([128, 512], F32, tag="b")
                    nc.tensor.matmul(rbps[:], ones128r[:], rrowr[:],
                                     start=True, stop=True)
                    rbc = bc_p.tile([128, 512], F32)
                    nc.scalar.copy(rbc[:], rbps[:])
                    for dt in range(4):
                        tmp = tmp_p.tile([128, 512], F32)
                        nc.vector.tensor_sub(tmp[:], yT[:, dt, cols].bitcast(F32),
                                             mbc[:])
                        nc.vector.scalar_tensor_tensor(
                            xT[:, dt, cols], tmp[:], lng[:, l, dt:dt + 1],
                            rbc[:], op0=ALU.mult, op1=ALU.mult)
                        if gates["lnb"]:
                            nc.vector.tensor_scalar_add(
                                xT[:, dt, cols], xT[:, dt, cols],
                                lnb[:, l, dt:dt + 1])

            # ---- final projection + tanh ----
            for qs in range(2):
                cols = bass.ts(qs, 512)
                fps = psS.tile([1, 512], F32, tag="s", padded_shape=None)
                for dt in range(4):
                    nc.tensor.matmul(fps[:], opw[:, dt:dt + 1], xT[:, dt, cols],
                                     start=(dt == 0), stop=(dt == 3))
                th = row_p.tile([1, 512], F32)
                nc.scalar.activation(th[:], fps[:], AF.Tanh, bias=opb[0:1, 0:1])
                orow = row_p.tile([1, 512], F32)
                nc.scalar.mul(orow[:], th[:], DELTA_SCALE)
                nc.sync.dma_start(E["out"][b:b + 1, cols], orow[:])


def _pack(W, L4shape=False):
    return W


def _host_pack(inputs):
    f32 = np.float32
    ip = {k: np.asarray(v, f32) for k, v in inputs.items()}
    B = ip["features"].shape[0]
    featT = np.ascontiguousarray(ip["features"].transpose(0, 2, 1))  # [B, IN, S]

    def packw(w):  # [L, D, D] (out,in) -> [128, L, 4, D] transposed lhsT layout
        return np.ascontiguousarray(
            w.transpose(0, 2, 1).reshape(L, 4, 128, D).transpose(2, 0, 1, 3))

    def packb(x):  # [L, D] -> [128, L, 4]
        return np.ascontiguousarray(x.reshape(L, 4, 128).transpose(2, 0, 1))

    wq = ip["in_proj_w"][:, :D, :]
    wk = ip["in_proj_w"][:, D:2 * D, :]
    wv = ip["in_proj_w"][:, 2 * D:, :]
    qbias = ip["in_proj_b"][:, :D]
    kbias = ip["in_proj_b"][:, D:2 * D]
    vbias = ip["in_proj_b"][:, 2 * D:]

    rows = np.arange(128)[:, None]
    colsi = np.arange(128)[None, :]
    maskA = np.where(rows > colsi, NEG, 0.0).astype(f32)

    consts = dict(
        hwT=np.ascontiguousarray(ip["hist_w"].T),
        qwT=np.ascontiguousarray(ip["query_w"].T),
        hb=np.ascontiguousarray(ip["hist_b"].reshape(4, 128).T),
        qb=np.ascontiguousarray(ip["query_b"].reshape(4, 128).T),
        wqT=packw(wq), wkT=packw(wk), wvT=packw(wv),
        woT=packw(ip["attn_out_w"]),
        qbl=packb(qbias), kbl=packb(kbias), vbl=packb(vbias),
        abl=packb(ip["attn_out_b"]),
        lng=packb(ip["ln_g"]), lnb=packb(ip["ln_b"]),
        opw=np.ascontiguousarray(ip["out_proj_w"].reshape(4, 128).T),
        opb=ip["out_proj_b"].reshape(1, 1),
        maskA=maskA,
        cones=np.ones((128, 128), f32),
    )
    gates = dict(vb=bool(np.any(vbias != 0.0)), lnb=bool(np.any(ip["ln_b"] != 0.0)))
    return featT, consts, gates, B


_CACHE = {}


def build_and_inmaps(inputs):
    featT, consts, gates, B = _host_pack(inputs)
    import os as _os
    key = (gates["vb"], gates["lnb"], _os.environ.get("KSTAGE", "9"))
    if key not in _CACHE:
        _CACHE[key] = _build(gates)
    nc = _CACHE[key]
    in_maps = []
    for c in range(NCORES):
        m = dict(consts)
        m["featT"] = np.ascontiguousarray(featT[c * BL:(c + 1) * BL])
        in_maps.append(m)
    return nc, in_maps


def kernel(**inputs):
    from concourse.bass_utils import run_bass_kernel_spmd
    nc, in_maps = build_and_inmaps(inputs)
    res = run_bass_kernel_spmd(nc, in_maps, list(range(NCORES)))
    outs = [res.results[c]["out"] for c in range(NCORES)]
    return np.concatenate(outs, axis=0).astype(np.float32)


# revision 7
# speedup vs baseline: 6.7523x; 6.7523x over previous
import sys
if "/opt/trn_rl_repo" not in sys.path:
    sys.path.insert(0, "/opt/trn_rl_repo")
import numpy as np
import concourse.bass as bass
from concourse import bacc
import concourse.tile as tile
from concourse import mybir

F32 = mybir.dt.float32
F32R = mybir.dt.float32r
BF16 = mybir.dt.bfloat16
AF = mybir.ActivationFunctionType
ALU = mybir.AluOpType

D = 512
H = 8
HD = 64
L = 2
IN = 16
S = 1024
BL = 2          # batch elems per core
NCORES = 8
LN_EPS = 1e-5
DELTA_SCALE = 1.5
NEG = -1.0e30


def _build(gates):
    """Build the per-core SPMD program. gates: dict of host-value-dependent flags."""
    import os as _os
    gates = dict(gates)
    gates["stage"] = int(_os.environ.get("KSTAGE", "9"))
    gates["reps"] = int(_os.environ.get("KREPS", "1"))
    nc = bacc.Bacc(None, target_bir_lowering=False, debug=False, num_devices=NCORES)

    dp = nc.declare_dram_parameter
    featT_e = dp("featT", [BL, IN, S], F32, isOutput=False)
    hwT_e = dp("hwT", [IN, D], F32, isOutput=False)
    qwT_e = dp("qwT", [IN, D], F32, isOutput=False)
    hb_e = dp("hb", [128, 4], F32, isOutput=False)
    qb_e = dp("qb", [128, 4], F32, isOutput=False)
    wqT_e = dp("wqT", [128, L, 4, D], F32, isOutput=False)
    wkT_e = dp("wkT", [128, L, 4, D], F32, isOutput=False)
    wvT_e = dp("wvT", [128, L, 4, D], F32, isOutput=False)
    woT_e = dp("woT", [128, L, 4, D], F32, isOutput=False)
    qbl_e = dp("qbl", [128, L, 4], F32, isOutput=False)
    kbl_e = dp("kbl", [128, L, 4], F32, isOutput=False)
    vbl_e = dp("vbl", [128, L, 4], F32, isOutput=False)
    abl_e = dp("abl", [128, L, 4], F32, isOutput=False)
    lng_e = dp("lng", [128, L, 4], F32, isOutput=False)
    lnb_e = dp("lnb", [128, L, 4], F32, isOutput=False)
    opw_e = dp("opw", [128, 4], F32, isOutput=False)
    opb_e = dp("opb", [1, 1], F32, isOutput=False)
    maskA_e = dp("maskA", [128, 128], F32, isOutput=False)
    cones_e = dp("cones", [128, 128], F32, isOutput=False)
    out_e = dp("out", [BL, S], F32, isOutput=True)

    with tile.TileContext(nc) as tc:
        _emit(nc, tc, gates, dict(
            featT=featT_e, hwT=hwT_e, qwT=qwT_e, hb=hb_e, qb=qb_e,
            wqT=wqT_e, wkT=wkT_e, wvT=wvT_e, woT=woT_e,
            qbl=qbl_e, kbl=kbl_e, vbl=vbl_e, abl=abl_e,
            lng=lng_e, lnb=lnb_e, opw=opw_e, opb=opb_e,
            maskA=maskA_e, cones=cones_e, out=out_e))
    nc.compile()
    return nc


def _emit(nc, tc, gates, E):
    from contextlib import ExitStack
    ctx = ExitStack()
    with ctx:
        P = bass.MemorySpace.PSUM
        wp = ctx.enter_context(tc.tile_pool(name="wp", bufs=1))
        feat_p = ctx.enter_context(tc.tile_pool(name="feat", bufs=1))
        hist_p = ctx.enter_context(tc.tile_pool(name="hist", bufs=1))
        x_p = ctx.enter_context(tc.tile_pool(name="x", bufs=1))
        y_p = ctx.enter_context(tc.tile_pool(name="y", bufs=1))
        q_p = ctx.enter_context(tc.tile_pool(name="q", bufs=1))
        k_p = ctx.enter_context(tc.tile_pool(name="k", bufs=1))
        v_p = ctx.enter_context(tc.tile_pool(name="v", bufs=1))
        pr_p = ctx.enter_context(tc.tile_pool(name="pr", bufs=1))
        o_p = ctx.enter_context(tc.tile_pool(name="o", bufs=1))
        x2_p = ctx.enter_context(tc.tile_pool(name="x2", bufs=1))
        tmp_p = ctx.enter_context(tc.tile_pool(name="tmp", bufs=1))
        bc_p = ctx.enter_context(tc.tile_pool(name="bc", bufs=1))
        sinv_p = ctx.enter_context(tc.tile_pool(name="sinv", bufs=1))
        row_p = ctx.enter_context(tc.tile_pool(name="row", bufs=1))
        psA = ctx.enter_context(tc.tile_pool(name="psA", bufs=2, space=P))
        psS = ctx.enter_context(tc.tile_pool(name="psS", bufs=2, space=P))
        psV = ctx.enter_context(tc.tile_pool(name="psV", bufs=2, space=P))
        psB = ctx.enter_context(tc.tile_pool(name="psB", bufs=2, space=P))

        # ---- persistent weights/consts ----
        hwT = wp.tile([IN, D], F32R)
        qwT = wp.tile([IN, D], F32R)
        hb = wp.tile([128, 4], F32)
        qb = wp.tile([128, 4], F32)
        wqT = wp.tile([128, L, 4, D], F32R)
        wkT = wp.tile([128, L, 4, D], F32R)
        wvT = wp.tile([128, L, 4, D], F32R)
        woT = wp.tile([128, L, 4, D], F32R)
        qbl = wp.tile([128, L, 4], F32)
        kbl = wp.tile([128, L, 4], F32)
        vbl = wp.tile([128, L, 4], F32)
        abl = wp.tile([128, L, 4], F32)
        lng = wp.tile([128, L, 4], F32)
        lnb = wp.tile([128, L, 4], F32)
        opw = wp.tile([128, 4], F32R)
        opb = wp.tile([1, 1], F32)
        maskA = wp.tile([128, 128], F32)
        ones64r = wp.tile([1, HD], F32R)    # bcast lhsT across 64 parts
        ones128c = wp.tile([128, 1], F32R)  # LN-sum lhsT
        ones128r = wp.tile([1, 128], F32R)  # bcast lhsT across 128 parts
        cones = wp.tile([128, HD], F32)     # f32 ones for v-aug column

        g = nc.gpsimd
        g.dma_start(hwT[:], E["hwT"][:])
        g.dma_start(qwT[:], E["qwT"][:])
        g.dma_start(hb[:], E["hb"][:])
        g.dma_start(qb[:], E["qb"][:])
        g.dma_start(wqT[:], E["wqT"][:])
        g.dma_start(wkT[:], E["wkT"][:])
        g.dma_start(wvT[:], E["wvT"][:])
        g.dma_start(woT[:], E["woT"][:])
        g.dma_start(qbl[:], E["qbl"][:])
        g.dma_start(kbl[:], E["kbl"][:])
        g.dma_start(vbl[:], E["vbl"][:])
        g.dma_start(abl[:], E["abl"][:])
        g.dma_start(lng[:], E["lng"][:])
        g.dma_start(lnb[:], E["lnb"][:])
        g.dma_start(opw[:], E["opw"][:])
        g.dma_start(opb[:], E["opb"][:])
        g.dma_start(maskA[:], E["maskA"][:])
        g.dma_start(ones64r[:], E["cones"][0:1, 0:HD])
        g.dma_start(ones128c[:], E["cones"][:, 0:1])
        g.dma_start(ones128r[:], E["cones"][0:1, :])
        g.dma_start(cones[:], E["cones"][:, 0:HD])

        for rep in range(gates["reps"]):
          for b in range(BL):
            featT = feat_p.tile([IN, S], F32R)
            g.dma_start(featT[:], E["featT"][b])

            histT = hist_p.tile([128, 4, S], F32R)
            xT = x_p.tile([128, 4, S], F32R)
            # input projections: histT/xT [d, t] = W[d,:] @ featT
            for dt in range(4):
                for qs in range(2):
                    cols = bass.ts(qs, 512)
                    ps = psA.tile([128, 512], F32, tag="a")
                    nc.tensor.matmul(ps[:], hwT[:, dt * 128:(dt + 1) * 128],
                                     featT[:, cols], start=True, stop=True)
                    nc.scalar.activation(histT[:, dt, cols], ps[:], AF.Identity,
                                         bias=hb[:, dt:dt + 1])
                    ps2 = psA.tile([128, 512], F32, tag="a")
                    nc.tensor.matmul(ps2[:], qwT[:, dt * 128:(dt + 1) * 128],
                                     featT[:, cols], start=True, stop=True)
                    nc.scalar.activation(xT[:, dt, cols], ps2[:], AF.Identity,
                                         bias=qb[:, dt:dt + 1])

            if gates["stage"] < 2:
                continue
            for l in range(L):
                # ---- q/k projections (transposed layout, bf16 out) ----
                qT = q_p.tile([128, 4, S], BF16)
                kT = k_p.tile([128, 4, S], BF16)
                for dt in range(4):
                    for qs in range(2):
                        cols = bass.ts(qs, 512)
                        ps = psA.tile([128, 512], F32, tag="a")
                        for kt in range(4):
                            nc.tensor.matmul(
                                ps[:], wqT[:, l, kt, dt * 128:(dt + 1) * 128],
                                xT[:, kt, cols], start=(kt == 0), stop=(kt == 3))
                        nc.scalar.activation(qT[:, dt, cols], ps[:], AF.Identity,
                                             bias=qbl[:, l, dt:dt + 1])
                        ps2 = psA.tile([128, 512], F32, tag="a")
                        for kt in range(4):
                            nc.tensor.matmul(
                                ps2[:], wkT[:, l, kt, dt * 128:(dt + 1) * 128],
                                histT[:, kt, cols], start=(kt == 0), stop=(kt == 3))
                        nc.scalar.activation(kT[:, dt, cols], ps2[:], AF.Identity,
                                             bias=kbl[:, l, dt:dt + 1])

                # ---- v projection (natural layout + ones column, bf16) ----
                vN = v_p.tile([128, 8, H, HD + 1], BF16)
                for tt in range(8):
                    ps = psA.tile([128, 512], F32, tag="a")
                    for kt in range(4):
                        nc.tensor.matmul(
                            ps[:], histT[:, kt, tt * 128:(tt + 1) * 128],
                            wvT[:, l, kt, :], start=(kt == 0), stop=(kt == 3))
                    for h in range(H):
                        nc.scalar.copy(vN[:, tt, h, 0:HD], ps[:, h * HD:(h + 1) * HD])
                nc.scalar.copy(
                    vN[:, :, :, HD].rearrange("p a b -> p (a b)"), cones[:])

                # ---- attention ----
                if gates["stage"] < 3:
                    continue
                outT = o_p.tile([128, 4, S], F32R)
                for h in range(H):
                    hp = (h % 2) * 64
                    dht = h // 2
                    for qblk in range(2):
                        probsT = pr_p.tile([128, 8, 512], BF16)
                        pv = psV.tile([65, 512], F32, tag="v")
                        nkj = 4 * (qblk + 1)
                        for kj in range(nkj):
                            off = max(0, (kj - 4 * qblk) * 128)
                            n = 512 - off
                            sc = psS.tile([128, 512], F32, tag="s")
                            nc.tensor.matmul(
                                sc[:, off:],
                                kT[hp:hp + 64, dht, kj * 128:(kj + 1) * 128],
                                qT[hp:hp + 64, dht, qblk * 512 + off:(qblk + 1) * 512],
                                start=True, stop=True)
                            if kj >= 4 * qblk:
                                nc.vector.tensor_add(sc[:, off:off + 128],
                                                     sc[:, off:off + 128], maskA[:])
                            nc.scalar.activation(probsT[:, kj, off:], sc[:, off:],
                                                 AF.Exp, scale=0.125)
                            nc.tensor.matmul(pv[:, off:], vN[:, kj, h, :],
                                             probsT[:, kj, off:],
                                             start=(kj == 0), stop=(kj == nkj - 1))
                        # normalize by softmax sums (row 64 of pv)
                        srow = row_p.tile([1, 512], F32R, bufs=2)
                        nc.scalar.copy(srow[:], pv[64:65, :])
                        sb = psB.tile([64, 512], F32, tag="b")
                        nc.tensor.matmul(sb[:], ones64r[:], srow[:],
                                         start=True, stop=True)
                        sinv = sinv_p.tile([64, 512], F32)
                        nc.vector.reciprocal(sinv[:], sb[:])
                        cols = bass.ts(qblk, 512)
                        nc.vector.tensor_mul(outT[hp:hp + 64, dht, cols],
                                             pv[0:64, :], sinv[:])
                        if gates["vb"]:
                            nc.vector.tensor_scalar_add(
                                outT[hp:hp + 64, dht, cols],
                                outT[hp:hp + 64, dht, cols],
                                vbl[hp:hp + 64, l, dht:dht + 1])

                # ---- attn out proj + residual add ----
                if gates["stage"] < 4:
                    continue
                yT = y_p.tile([128, 4, S], F32R)
                for dt in range(4):
                    for qs in range(2):
                        cols = bass.ts(qs, 512)
                        ps = psA.tile([128, 512], F32, tag="a")
                        for kt in range(4):
                            nc.tensor.matmul(
                                ps[:], woT[:, l, kt, dt * 128:(dt + 1) * 128],
                                outT[:, kt, cols], start=(kt == 0), stop=(kt == 3))
                        nc.vector.scalar_tensor_tensor(
                            yT[:, dt, cols], ps[:], abl[:, l, dt:dt + 1],
                            xT[:, dt, cols].bitcast(F32),
                            op0=ALU.add, op1=ALU.add)

                # ---- layernorm ----
                xT = x_p.tile([128, 4, S], F32R)
                for qs in range(2):
                    cols = bass.ts(qs, 512)
                    mps = psS.tile([1, 512], F32, tag="s", padded_shape=None)
                    for dt in range(4):
                        nc.tensor.matmul(mps[:], ones128c[:], yT[:, dt, cols],
                                         start=(dt == 0), stop=(dt == 3))
                    vps = psS.tile([1, 512], F32, tag="s", padded_shape=None)
                    for dt in range(4):
                        x2 = x2_p.tile([128, 512], F32R)
                        nc.scalar.activation(x2[:], yT[:, dt, cols].bitcast(F32),
                                             AF.Square)
                        nc.tensor.matmul(vps[:], ones128c[:], x2[:],
                                         start=(dt == 0), stop=(dt == 3))
                    mrow = row_p.tile([1, 512], F32R)
                    nc.scalar.mul(mrow[:], mps[:], 1.0 / D)
                    s1 = row_p.tile([1, 512], F32)
                    nc.vector.tensor_mul(s1[:], mrow[:].bitcast(F32),
                                         mrow[:].bitcast(F32))
                    s2 = row_p.tile([1, 512], F32)
                    nc.vector.scalar_tensor_tensor(
                        s2[:], vps[:], 1.0 / D, s1[:],
                        op0=ALU.mult, op1=ALU.subtract)
                    s4 = row_p.tile([1, 512], F32)
                    nc.vector.tensor_scalar_add(s4[:], s2[:], LN_EPS)
                    s3 = row_p.tile([1, 512], F32)
                    nc.scalar.sqrt(s3[:], s4[:])
                    rrowr = row_p.tile([1, 512], F32R)
                    with nc.allow_low_precision(reason="f32r rstd"):
                        nc.vector.reciprocal(rrowr[:], s3[:])
                    # broadcast mean and rstd to 128 partitions
                    mbps = psB.tile([128, 512], F32, tag="b")
                    nc.tensor.matmul(mbps[:], ones128r[:], mrow[:],
                                     start=True, stop=True)
                    mbc = bc_p.tile([128, 512], F32)
                    nc.scalar.copy(mbc[:], mbps[:])
                    rbps = psB.tile([128, 512], F32, tag="b")
                    nc.tensor.matmul(rbps[:], ones128r[:], rrowr[:],
                                     start=True, stop=True)
                    rbc = bc_p.tile([128, 512], F32)
                    nc.scalar.copy(rbc[:], rbps[:])
                    for dt in range(4):
                        tmp = tmp_p.tile([128, 512], F32)
                        nc.vector.tensor_sub(tmp[:], yT[:, dt, cols].bitcast(F32),
                                             mbc[:])
                        nc.vector.scalar_tensor_tensor(
                            xT[:, dt, cols], tmp[:], lng[:, l, dt:dt + 1],
                            rbc[:], op0=ALU.mult, op1=ALU.mult)
                        if gates["lnb"]:
                            nc.vector.tensor_scalar_add(
                                xT[:, dt, cols], xT[:, dt, cols],
                                lnb[:, l, dt:dt + 1])

            # ---- final projection + tanh ----
            for qs in range(2):
                cols = bass.ts(qs, 512)
                fps = psS.tile([1, 512], F32, tag="s", padded_shape=None)
                for dt in range(4):
                    nc.tensor.matmul(fps[:], opw[:, dt:dt + 1], xT[:, dt, cols],
                                     start=(dt == 0), stop=(dt == 3))
                th = row_p.tile([1, 512], F32)
                nc.scalar.activation(th[:], fps[:], AF.Tanh, bias=opb[0:1, 0:1])
                orow = row_p.tile([1, 512], F32)
                nc.scalar.mul(orow[:], th[:], DELTA_SCALE)
                nc.sync.dma_start(E["out"][b:b + 1, cols], orow[:])


def _pack(W, L4shape=False):
    return W


def _host_pack(inputs):
    f32 = np.float32
    ip = {k: np.asarray(v, f32) for k, v in inputs.items()}
    B = ip["features"].shape[0]
    featT = np.ascontiguousarray(ip["features"].transpose(0, 2, 1))  # [B, IN, S]

    def packw(w):  # [L, D, D] (out,in) -> [128, L, 4, D] transposed lhsT layout
        return np.ascontiguousarray(
            w.transpose(0, 2, 1).reshape(L, 4, 128, D).transpose(2, 0, 1, 3))

    def packb(x):  # [L, D] -> [128, L, 4]
        return np.ascontiguousarray(x.reshape(L, 4, 128).transpose(2, 0, 1))

    wq = ip["in_proj_w"][:, :D, :]
    wk = ip["in_proj_w"][:, D:2 * D, :]
    wv = ip["in_proj_w"][:, 2 * D:, :]
    qbias = ip["in_proj_b"][:, :D]
    kbias = ip["in_proj_b"][:, D:2 * D]
    vbias = ip["in_proj_b"][:, 2 * D:]

    rows = np.arange(128)[:, None]
    colsi = np.arange(128)[None, :]
    maskA = np.where(rows > colsi, NEG, 0.0).astype(f32)

    consts = dict(
        hwT=np.ascontiguousarray(ip["hist_w"].T),
        qwT=np.ascontiguousarray(ip["query_w"].T),
        hb=np.ascontiguousarray(ip["hist_b"].reshape(4, 128).T),
        qb=np.ascontiguousarray(ip["query_b"].reshape(4, 128).T),
        wqT=packw(wq), wkT=packw(wk), wvT=packw(wv),
        woT=packw(ip["attn_out_w"]),
        qbl=packb(qbias), kbl=packb(kbias), vbl=packb(vbias),
        abl=packb(ip["attn_out_b"]),
        lng=packb(ip["ln_g"]), lnb=packb(ip["ln_b"]),
        opw=np.ascontiguousarray(ip["out_proj_w"].reshape(4, 128).T),
        opb=ip["out_proj_b"].reshape(1, 1),
        maskA=maskA,
        cones=np.ones((128, 128), f32),
    )
    gates = dict(vb=bool(np.any(vbias != 0.0)), lnb=bool(np.any(ip["ln_b"] != 0.0)))
    return featT, consts, gates, B


_CACHE = {}


def build_and_inmaps(inputs):
    featT, consts, gates, B = _host_pack(inputs)
    import os as _os
    key = (gates["vb"], gates["lnb"], _os.environ.get("KSTAGE", "9"), _os.environ.get("KREPS", "1"))
    if key not in _CACHE:
        _CACHE[key] = _build(gates)
    nc = _CACHE[key]
    in_maps = []
    for c in range(NCORES):
        m = dict(consts)
        m["featT"] = np.ascontiguousarray(featT[c * BL:(c + 1) * BL])
        in_maps.append(m)
    return nc, in_maps


def kernel(**inputs):
    from concourse.bass_utils import run_bass_kernel_spmd
    nc, in_maps = build_and_inmaps(inputs)
    res = run_bass_kernel_spmd(nc, in_maps, list(range(NCORES)))
    outs = [res.results[c]["out"] for c in range(NCORES)]
    return np.concatenate(outs, axis=0).astype(np.float32)
